# revision 12
# baseline (speedup 1.0000x reference)
"""Trainium2 Bass kernel for nn_MARLCommunicationLayer (N=4096 agents, D=256, H=4, L=2).

Self-contained: builds an 8-core SPMD Bass/Tile program, shards inputs
(destination-block row partition of N), runs via run_bass_kernel_spmd, and
gathers the full output.

Algorithm notes (vs the dense reference):
- top-k=5 adjacency via per-row 5th-largest threshold (nc.vector.max gives
  the top-8 per row); symmetric mask[i,j] = sim[i,j] >= min(thr_i, thr_j).
- GAT softmax without any exp over [N,N]:
  exp(leakyrelu(z)) = [z>0]*es_i*ed_j + [z<=0]*fs_i*fd_j with
  es=exp(a_s), fs=exp(0.2 a_s), ed=exp(a_d), fd=exp(0.2 a_d).
  B1[i,j] = mask*[z>0], B2[i,j] = mask*[z<=0] are built in one DVE
  scalar_tensor_tensor each, then contracted on the PE against
  es/fs-scaled features (ones-column augmented for the denominators).
- MHA pooling: per-head S^T = k_chunk @ q_own^T on PE, exp on ACT
  (tiny logits -> no max subtraction), ctx^T accumulated with a
  ones-augmented v to get denominators for free.
"""

import contextlib
import ctypes
import math
import sys
import types

import numpy as np

# ---------------------------------------------------------------------------
# antenv.axon_hooks shim: the agent image lacks this module but
# concourse.bass_utils imports it when trace=True under axon.
# ---------------------------------------------------------------------------
if "antenv.axon_hooks" not in sys.modules:
    _HOOK = [None]
    _m = types.ModuleType("antenv.axon_hooks")
    _m.set_axon_ntff_profile_hook = lambda h: _HOOK.__setitem__(0, h)
    _m.get_axon_ntff_profile_hook = lambda: _HOOK[0]
    sys.modules["antenv.axon_hooks"] = _m


def install_ntff_hook(so_path="/opt/axon/libaxon_pjrt.so"):
    import antenv.axon_hooks as ah

    if ah.get_axon_ntff_profile_hook() is not None:
        return
    try:
        lib = ctypes.CDLL(so_path)
    except OSError:
        return
    if not hasattr(lib, "axon_start_nrt_profile"):
        return
    lib.axon_start_nrt_profile.argtypes = [
        ctypes.POINTER(ctypes.c_int64),
        ctypes.c_size_t,
    ]
    lib.axon_start_nrt_profile.restype = ctypes.c_int64
    lib.axon_stop_nrt_profile.argtypes = [ctypes.c_char_p]
    lib.axon_stop_nrt_profile.restype = ctypes.c_int64

    @contextlib.contextmanager
    def _hook(output_dir, device_ids):
        import jax

        jax.devices()
        if device_ids:
            ids = (ctypes.c_int64 * len(device_ids))(*device_ids)
            rc = lib.axon_start_nrt_profile(ids, len(device_ids))
        else:
            rc = lib.axon_start_nrt_profile(None, 0)
        if rc != 0:
            raise RuntimeError(f"axon_start_nrt_profile rc={rc}")
        try:
            yield
        finally:
            n = lib.axon_stop_nrt_profile(str(output_dir).encode())
            print(f"profile: {n} file(s) written to {output_dir}")

    ah.set_axon_ntff_profile_hook(_hook)


import concourse.bass as bass
import concourse.bacc as bacc
import concourse.bass_utils as bass_utils
import concourse.mybir as mybir
import concourse.tile as tile
from concourse.masks import make_identity

# zero-egress container: no S3 uploads
bass_utils.upload_artifacts = lambda tmpdir: "(local)"

F32 = mybir.dt.float32
BF16 = mybir.dt.bfloat16
AF = mybir.ActivationFunctionType
ALU = mybir.AluOpType

N, D, H, L, MSG = 4096, 256, 4, 2, 32
C = D // H  # 64
NCORES = 8
BIG = 1.0e9


def bcast_ap(src: bass.AP, parts: int) -> bass.AP:
    """Replicate a flat/row access pattern across `parts` partitions."""
    free = [list(d) for d in src.ap if d[1] != 1] or [[1, 1]]
    return bass.AP(tensor=src.tensor, offset=src.offset, ap=[[0, parts]] + free)


def free_bcast(src: bass.AP, reps: int) -> bass.AP:
    """[P, F] -> [P, F, reps] with stride-0 inner dim."""
    return bass.AP(tensor=src.tensor, offset=src.offset,
                   ap=[list(d) for d in src.ap] + [[0, reps]])


def build_program(n=N, ncores=NCORES):
    NB = n // 128          # source chunks
    NO = n // ncores       # own rows per core
    OB = NO // 128         # own chunks
    assert NO % 128 == 0 and NO <= 512

    nc = bacc.Bacc(
        "TRN2",
        target_bir_lowering=False,
        debug=False,
        enable_asserts=False,
        num_devices=ncores,
    )

    def din(name, shape):
        return nc.dram_tensor(name, list(shape), F32, kind="ExternalInput").ap()

    states = din("agent_states", (n, D))
    emb = din("agent_emb", (n, D))
    role = din("role_emb", (n, D // 4))
    gat_W = din("gat_W", (L, D, D))
    gat_att_src = din("gat_att_src", (L, H, C))
    gat_att_dst = din("gat_att_dst", (L, H, C))
    gat_bias = din("gat_bias", (L, D))
    ln_gamma = din("ln_gamma", (L, D))
    ln_beta = din("ln_beta", (L, D))
    enc_W1 = din("enc_W1", (D, 2 * MSG))
    enc_b1 = din("enc_b1", (2 * MSG,))
    enc_W2 = din("enc_W2", (2 * MSG, MSG))
    enc_b2 = din("enc_b2", (MSG,))
    dec_W1 = din("dec_W1", (MSG, D // 2))
    dec_b1 = din("dec_b1", (D // 2,))
    dec_W2 = din("dec_W2", (D // 2, D))
    dec_b2 = din("dec_b2", (D,))
    mha_in_w = din("mha_in_w", (D, 3 * D))
    mha_in_b = din("mha_in_b", (3 * D,))
    mha_out_w = din("mha_out_w", (D, D))
    mha_out_b = din("mha_out_b", (D,))
    proj_W = din("proj_W", (D, D))
    proj_b = din("proj_b", (D,))
    gate_W1 = din("gate_W1", (2 * D, D))
    gate_b1 = din("gate_b1", (D,))
    gate_W2 = din("gate_W2", (D, 1))
    gate_b2 = din("gate_b2", (1,))
    states_own = din("states_own", (NO, D))
    emb_own = din("emb_own", (NO, D))
    role_own = din("role_own", (NO, D // 4))

    out_own = nc.dram_tensor("out_own", [NO, D], F32, kind="ExternalOutput").ap()

    rg = [list(range(ncores))]

    with tile.TileContext(nc) as tc, contextlib.ExitStack() as top:
        consts = top.enter_context(tc.tile_pool(name="consts", bufs=1))
        persist = top.enter_context(tc.tile_pool(name="persist", bufs=1))
        dram = top.enter_context(tc.tile_pool(name="dram", bufs=1, space="DRAM"))

        id_f32 = consts.tile([128, 128], F32, name="id_f32")
        make_identity(nc, id_f32)
        id_bf = consts.tile([128, 128], BF16, name="id_bf")
        make_identity(nc, id_bf)
        eps_ln = consts.tile([128, 1], F32, name="eps_ln")
        nc.vector.memset(eps_ln, 1e-5)

        def transpose_into(dst_ap, src_ap, psum_pool, dtype, tag="tr"):
            """dst[128, 128] = src[128, 128].T via PE + DVE copy."""
            ident = id_f32 if dtype == F32 else id_bf
            pt = psum_pool.tile([128, 128], dtype, name=f"{tag}_ps", tag=f"{tag}_ps")
            nc.tensor.transpose(pt, src_ap, ident)
            nc.vector.tensor_copy(dst_ap, pt)

        # persistent across stages; xT gets its own stack so stage D can
        # release it after the last consumer (message-chain matmuls)
        xT_stack = contextlib.ExitStack()
        xT_pool = xT_stack.enter_context(
            tc.tile_pool(name="xTp", bufs=1, side="right"))
        enhT = [xT_pool.tile([128, n], F32, name=f"enhT{k}", tag=f"enhT{k}")
                for k in range(2)]
        x_own = [persist.tile([128, D], F32, name=f"x_own{j}", tag=f"x_own{j}")
                 for j in range(OB)]
        x_ownT = [persist.tile([128, NO], F32, name=f"x_ownT{k}", tag=f"x_ownT{k}")
                  for k in range(2)]
        states_ownT = [persist.tile([128, NO], F32, name=f"st_ownT{k}",
                                    tag=f"st_ownT{k}") for k in range(2)]

        xT = enhT  # current-layer transposed activations, rebuilt in place

        with contextlib.ExitStack() as mid:  # scope for mask + nsT
            maskp = mid.enter_context(tc.tile_pool(name="maskp", bufs=1))
            maskBIG = [maskp.tile([128, NO], BF16, name=f"maskBIG{b}",
                                  tag=f"maskBIG{b}") for b in range(NB)]

            # =============================================================
            # Stage A: enhanced, ns, transposes
            # =============================================================
            with contextlib.ExitStack() as ab:
                stA = ab.enter_context(tc.tile_pool(name="stA", bufs=3))
                stA_ps = ab.enter_context(
                    tc.tile_pool(name="stA_ps", bufs=2, space="PSUM"))
                nsT_pool = ab.enter_context(tc.tile_pool(name="nsTp", bufs=1))

                nsT = [nsT_pool.tile([128, n], F32, name=f"nsT{k}", tag=f"nsT{k}")
                       for k in range(2)]
                ns_ownT = [nsT_pool.tile([128, NO], F32, name=f"ns_ownT{k}",
                                         tag=f"ns_ownT{k}") for k in range(2)]

                def enh_pipeline(st_ap, em_ap, ro_ap, b, enhT_dst, nsT_dst,
                                 keep_enh_into=None):
                    e = stA.tile([128, D], F32, name="e", tag="A_e")
                    t2 = stA.tile([128, D], F32, name="t2", tag="A_t2")
                    nc.sync.dma_start(out=e, in_=st_ap)
                    nc.sync.dma_start(out=t2, in_=em_ap)
                    nc.vector.tensor_add(e, e, t2)
                    ro = stA.tile([128, D // 4], F32, name="ro", tag="A_ro")
                    nc.sync.dma_start(out=ro, in_=ro_ap)
                    ro_rep = bass.AP(tensor=ro.tensor, offset=ro.offset,
                                     ap=[list(ro.ap[0]), [0, 4], [1, D // 4]])
                    e4 = bass.AP(tensor=e.tensor, offset=e.offset,
                                 ap=[list(e.ap[0]), [D // 4, 4], [1, D // 4]])
                    nc.vector.tensor_add(e4, e4, ro_rep)
                    if keep_enh_into is not None:
                        nc.vector.tensor_copy(keep_enh_into, e)
                    sq = stA.tile([128, D], F32, name="sq", tag="A_sq")
                    ss = stA.tile([128, 1], F32, name="ss", tag="A_ss")
                    nc.scalar.activation(sq, e, AF.Square, accum_out=ss)
                    nrm = stA.tile([128, 1], F32, name="nrm", tag="A_nrm")
                    nc.scalar.sqrt(nrm, ss)
                    nc.vector.tensor_scalar_max(nrm, nrm, 1e-12)
                    rn = stA.tile([128, 1], F32, name="rn", tag="A_rn")
                    nc.vector.reciprocal(rn, nrm)
                    nsb = stA.tile([128, D], F32, name="nsb", tag="A_nsb")
                    nc.scalar.activation(nsb, e, AF.Copy, scale=rn)
                    for k in range(2):
                        transpose_into(enhT_dst[k][:, b * 128:(b + 1) * 128],
                                       e[:, k * 128:(k + 1) * 128], stA_ps, F32)
                        transpose_into(nsT_dst[k][:, b * 128:(b + 1) * 128],
                                       nsb[:, k * 128:(k + 1) * 128], stA_ps, F32)

                for b in range(NB):
                    enh_pipeline(states[b * 128:(b + 1) * 128],
                                 emb[b * 128:(b + 1) * 128],
                                 role[b * 128:(b + 1) * 128], b, enhT, nsT)
                for j in range(OB):
                    enh_pipeline(states_own[j * 128:(j + 1) * 128],
                                 emb_own[j * 128:(j + 1) * 128],
                                 role_own[j * 128:(j + 1) * 128], j,
                                 x_ownT, ns_ownT, keep_enh_into=x_own[j])
                    sto = stA.tile([128, D], F32, name="sto", tag="A_sto")
                    nc.sync.dma_start(out=sto,
                                      in_=states_own[j * 128:(j + 1) * 128])
                    for k in range(2):
                        transpose_into(states_ownT[k][:, j * 128:(j + 1) * 128],
                                       sto[:, k * 128:(k + 1) * 128], stA_ps, F32)

                # =============================================================
                # Stage B: sim rows -> thr; AllGather thr; mask columns
                # =============================================================
                thr_dram = dram.tile([NO], F32, name="thr_dram")
                thr_all = dram.tile([n], F32, name="thr_all", addr_space="Shared")

                with contextlib.ExitStack() as bb:
                    stB = bb.enter_context(tc.tile_pool(name="stB", bufs=2))
                    stB_ps = bb.enter_context(
                        tc.tile_pool(name="stB_ps", bufs=2, space="PSUM"))
                    thr_cols_own = stB.tile([128, OB], F32, name="thr_cols_own",
                                            tag="thrown", bufs=1)
                    for ob in range(OB):
                        simrow = stB.tile([128, n], F32, name="simrow",
                                          tag="B_simrow")
                        for jb in range(n // 512):
                            ps = stB_ps.tile([128, 512], F32, name="B_ps",
                                             tag="B_ps")
                            for k in range(2):
                                nc.tensor.matmul(
                                    ps,
                                    ns_ownT[k][:, ob * 128:(ob + 1) * 128],
                                    nsT[k][:, jb * 512:(jb + 1) * 512],
                                    start=(k == 0), stop=(k == 1))
                            nc.scalar.copy(simrow[:, jb * 512:(jb + 1) * 512], ps)
                        top8 = stB.tile([128, 8], F32, name="top8", tag="B_top8")
                        nc.vector.max(out=top8, in_=simrow)
                        nc.vector.tensor_copy(thr_cols_own[:, ob:ob + 1],
                                              top8[:, 4:5])
                    nc.sync.dma_start(
                        out=thr_dram[:].rearrange("(o p) -> p o", p=128),
                        in_=thr_cols_own)
                    nc.gpsimd.collective_compute(
                        "AllGather", ALU.bypass, replica_groups=rg,
                        ins=[thr_dram[:]], outs=[thr_all[:]])

                    thr_cols = stB.tile([128, NB], F32, name="thr_cols",
                                        tag="thrall", bufs=1)
                    nc.sync.dma_start(
                        out=thr_cols,
                        in_=thr_all[:].rearrange("(b p) -> p b", p=128))
                    thr_own_rep = stB.tile([128, NO], F32, name="thr_own_rep",
                                           tag="thrrep", bufs=1)
                    nc.sync.dma_start(out=thr_own_rep,
                                      in_=bcast_ap(thr_dram[:], 128))

                    for b in range(NB):
                        ps = stB_ps.tile([128, NO], F32, name="B_mps",
                                         tag="B_ps")
                        for jb in range((NO + 511) // 512):
                            w = min(512, NO - jb * 512)
                            for k in range(2):
                                nc.tensor.matmul(
                                    ps[:, jb * 512:jb * 512 + w],
                                    nsT[k][:, b * 128:(b + 1) * 128],
                                    ns_ownT[k][:, jb * 512:jb * 512 + w],
                                    start=(k == 0), stop=(k == 1))
                        cmask = stB.tile([128, NO], BF16, name="cmask",
                                         tag="B_cmask")
                        # 1.0 where NOT selected: min(thr_i, thr_j) > sim
                        nc.vector.scalar_tensor_tensor(
                            out=cmask, in0=thr_own_rep,
                            scalar=thr_cols[:, b:b + 1],
                            in1=ps, op0=ALU.min, op1=ALU.is_gt)
                        nc.vector.tensor_scalar_mul(maskBIG[b], cmask, BIG)

            # =============================================================
            # Stage C: GAT layers
            # =============================================================
            for l in range(L):
                with contextlib.ExitStack() as lst:
                    stC = lst.enter_context(tc.tile_pool(name=f"stC{l}", bufs=2))
                    lay = lst.enter_context(tc.tile_pool(name=f"lay{l}", bufs=1))

                    W_sb = [lay.tile([128, D], F32, name=f"W{k}", tag=f"W{k}")
                            for k in range(2)]
                    for k in range(2):
                        nc.sync.dma_start(out=W_sb[k],
                                          in_=gat_W[l, k * 128:(k + 1) * 128, :])
                    A_bd = [lay.tile([128, 8], BF16, name=f"Abd{k}", tag=f"Abd{k}")
                            for k in range(2)]
                    for k in range(2):
                        nc.vector.memset(A_bd[k], 0.0)
                    for h in range(H):
                        k, r = divmod(h * C, 128)
                        nc.gpsimd.dma_start(out=A_bd[k][r:r + C, h:h + 1],
                                            in_=gat_att_src[l, h, :])
                        nc.gpsimd.dma_start(out=A_bd[k][r:r + C, 4 + h:5 + h],
                                            in_=gat_att_dst[l, h, :])
                    bias_rep = lay.tile([128, D], F32, name="bias_rep",
                                        tag="bias_rep")
                    nc.sync.dma_start(out=bias_rep,
                                      in_=bcast_ap(gat_bias[l, :], 128))
                    gamma_rep = lay.tile([128, D], F32, name="gamma_rep",
                                         tag="gamma_rep")
                    nc.sync.dma_start(out=gamma_rep,
                                      in_=bcast_ap(ln_gamma[l, :], 128))
                    beta_rep = lay.tile([128, D], F32, name="beta_rep",
                                        tag="beta_rep")
                    nc.sync.dma_start(out=beta_rep,
                                      in_=bcast_ap(ln_beta[l, :], 128))

                    xsT = [lay.tile([128, n], BF16, name=f"xsT{k}",
                                    tag=f"xsT{k}") for k in range(2)]
                    xs_nat = [lay.tile([128, D + 1], BF16, name=f"xsn{b}",
                                       tag=f"xsn{b}") for b in range(NB)]
                    asb = lay.tile([128, NB, 8], F32, name="asb", tag="asb")
                    escols = lay.tile([128, NB, H], F32, name="escols",
                                      tag="escols")
                    fscols = lay.tile([128, NB, H], F32, name="fscols",
                                      tag="fscols")
                    ys1 = [lay.tile([128, H, C + 1], BF16, name=f"ys1_{b}",
                                    tag=f"ys1_{b}") for b in range(NB)]
                    ys2 = [lay.tile([128, H, C + 1], BF16, name=f"ys2_{b}",
                                    tag=f"ys2_{b}") for b in range(NB)]
                    a_dram = dram.tile([8, NO], F32, name=f"a_dram{l}")
                    a_dram_bf = dram.tile([8, NO], BF16, name=f"a_dram_bf{l}")

                    with tc.tile_pool(name=f"C14ps{l}", bufs=2,
                                      space="PSUM") as cps_pool:
                        # C1: xsT = (x @ W)^T, bf16
                        for m in range(2):
                            for jb in range(n // 512):
                                ps = cps_pool.tile([128, 512], F32, name="C1ps",
                                                   tag="Cps")
                                for k in range(2):
                                    nc.tensor.matmul(
                                        ps, W_sb[k][:, m * 128:(m + 1) * 128],
                                        xT[k][:, jb * 512:(jb + 1) * 512],
                                        start=(k == 0), stop=(k == 1))
                                nc.scalar.copy(
                                    xsT[m][:, jb * 512:(jb + 1) * 512], ps)
                        # C2: xs natural (bf16) + ones column
                        for b in range(NB):
                            nc.vector.memset(xs_nat[b][:, D:D + 1], 1.0)
                            for m in range(2):
                                transpose_into(
                                    xs_nat[b][:, m * 128:(m + 1) * 128],
                                    xsT[m][:, b * 128:(b + 1) * 128],
                                    cps_pool, BF16, tag="C2")
                        # C3: a-columns for all sources + exp
                        for b in range(NB):
                            ps = cps_pool.tile([128, 8], F32, name="C3ps",
                                               tag="Cps8")
                            for k in range(2):
                                nc.tensor.matmul(
                                    ps, xsT[k][:, b * 128:(b + 1) * 128],
                                    A_bd[k], start=(k == 0), stop=(k == 1))
                            nc.vector.tensor_copy(asb[:, b, :], ps)
                        nc.scalar.activation(escols, asb[:, :, 0:H], AF.Exp)
                        nc.scalar.activation(fscols, asb[:, :, 0:H], AF.Exp,
                                             scale=0.2)

                        # C4: own-block a^T; a_d broadcast + ed/fd columns
                        xs_ownT = [stC.tile([128, NO], BF16, name=f"xsoT{k}",
                                            tag=f"xsoT{k}", bufs=1)
                                   for k in range(2)]
                        for k in range(2):
                            ps = cps_pool.tile([128, NO], F32, name="C4ps",
                                               tag="Cps")
                            for jb in range((NO + 511) // 512):
                                w = min(512, NO - jb * 512)
                                for kk in range(2):
                                    nc.tensor.matmul(
                                        ps[:, jb * 512:jb * 512 + w],
                                        W_sb[kk][:, k * 128:(k + 1) * 128],
                                        x_ownT[kk][:, jb * 512:jb * 512 + w],
                                        start=(kk == 0), stop=(kk == 1))
                            nc.scalar.copy(xs_ownT[k], ps)
                        ps8 = cps_pool.tile([8, NO], F32, name="C4ps8",
                                            tag="Cps8")
                        for k in range(2):
                            nc.tensor.matmul(ps8, A_bd[k], xs_ownT[k],
                                             start=(k == 0), stop=(k == 1))
                        a_ownT = stC.tile([8, NO], F32, name="a_ownT",
                                          tag="a_ownT", bufs=1)
                        nc.vector.tensor_copy(a_ownT, ps8)
                        a_ownT_bf = stC.tile([8, NO], BF16, name="a_ownT_bf",
                                             tag="a_ownT_bf", bufs=1)
                        nc.vector.tensor_copy(a_ownT_bf, ps8)
                        nc.sync.dma_start(out=a_dram[:], in_=a_ownT)
                        nc.sync.dma_start(out=a_dram_bf[:], in_=a_ownT_bf)
                        a_d_rep = [stC.tile([128, NO], BF16, name=f"adrep{h}",
                                            tag=f"adrep{h}", bufs=1)
                                   for h in range(H)]
                        for h in range(H):
                            nc.sync.dma_start(
                                out=a_d_rep[h],
                                in_=bcast_ap(a_dram_bf[4 + h, :], 128))
                        a_own_cols = stC.tile([128, OB, 8], F32,
                                              name="a_own_cols", tag="aocols",
                                              bufs=1)
                        for o in range(OB):
                            src = bass.AP(tensor=a_dram[:].tensor,
                                          offset=a_dram[:].offset + o * 128,
                                          ap=[[1, 128], [NO, 8]])
                            nc.sync.dma_start(out=a_own_cols[:, o, :], in_=src)
                        edfd = stC.tile([128, OB, 8], F32, name="edfd",
                                        tag="edfd", bufs=1)
                        nc.scalar.activation(edfd[:, :, 0:4],
                                             a_own_cols[:, :, 4:8], AF.Exp)
                        nc.scalar.activation(edfd[:, :, 4:8],
                                             a_own_cols[:, :, 4:8], AF.Exp,
                                             scale=0.2)

                        # C5: ys tiles [128, H, C+1] bf16 = (es*xs | es)
                        for b in range(NB):
                            esb = escols[:, b, :]
                            fsb = fscols[:, b, :]
                            xs_v = xs_nat[b][:, 0:D].rearrange(
                                "p (h c) -> p h c", h=H)
                            nc.vector.tensor_mul(ys1[b][:, :, 0:C], xs_v,
                                                 free_bcast(esb, C))
                            nc.vector.tensor_copy(ys1[b][:, :, C], esb)
                            nc.vector.tensor_mul(ys2[b][:, :, 0:C], xs_v,
                                                 free_bcast(fsb, C))
                            nc.vector.tensor_copy(ys2[b][:, :, C], fsb)

                    # C6/C7: per-head contraction + combine
                    xnew = [stC.tile([128, D], F32, name=f"xnew{j}",
                                     tag=f"xnew{j}", bufs=1) for j in range(OB)]
                    with tc.tile_pool(name=f"gat{l}_ps", bufs=1,
                                      space="PSUM") as gps, \
                         tc.tile_pool(name=f"gat{l}_b", bufs=3) as bpool:
                        for h in range(H):
                            o1 = [gps.tile([128, C + 1], F32, name=f"o1_{j}",
                                           tag=f"o1_{j}") for j in range(OB)]
                            o2 = [gps.tile([128, C + 1], F32, name=f"o2_{j}",
                                           tag=f"o2_{j}") for j in range(OB)]
                            for b in range(NB):
                                B1 = bpool.tile([128, NO], BF16, name="B1",
                                                tag="B1")
                                B2 = bpool.tile([128, NO], BF16, name="B2",
                                                tag="B2")
                                nc.vector.scalar_tensor_tensor(
                                    out=B1, in0=a_d_rep[h],
                                    scalar=asb[:, b, h:h + 1],
                                    in1=maskBIG[b], op0=ALU.add, op1=ALU.is_gt)
                                nc.vector.scalar_tensor_tensor(
                                    out=B2, in0=B1, scalar=1.0,
                                    in1=maskBIG[b], op0=ALU.is_lt,
                                    op1=ALU.is_gt)
                                for j in range(OB):
                                    nc.tensor.matmul(
                                        o1[j], B1[:, j * 128:(j + 1) * 128],
                                        ys1[b][:, h, :],
                                        start=(b == 0), stop=(b == NB - 1))
                                    nc.tensor.matmul(
                                        o2[j], B2[:, j * 128:(j + 1) * 128],
                                        ys2[b][:, h, :],
                                        start=(b == 0), stop=(b == NB - 1))
                            for j in range(OB):
                                t = stC.tile([128, C + 1], F32, name="cmb_t",
                                             tag="cmb_t")
                                nc.vector.tensor_scalar_mul(
                                    t, o2[j], edfd[:, j, 4 + h:5 + h])
                                num = stC.tile([128, C + 1], F32, name="cmb_n",
                                               tag="cmb_n")
                                nc.vector.scalar_tensor_tensor(
                                    out=num, in0=o1[j],
                                    scalar=edfd[:, j, h:h + 1],
                                    in1=t, op0=ALU.mult, op1=ALU.add)
                                rec = stC.tile([128, 1], F32, name="cmb_r",
                                               tag="cmb_r")
                                nc.vector.reciprocal(rec, num[:, C:C + 1])
                                nc.vector.tensor_scalar_mul(
                                    xnew[j][:, h * C:(h + 1) * C],
                                    num[:, 0:C], rec)

                    # C8: bias + LN + residual (into x_own)
                    for j in range(OB):
                        nc.vector.tensor_add(xnew[j], xnew[j], bias_rep)
                        stats = stC.tile([128, 6], F32, name="ln_st",
                                         tag="ln_st")
                        nc.vector.bn_stats(out=stats, in_=xnew[j])
                        mv = stC.tile([128, 2], F32, name="ln_mv", tag="ln_mv")
                        nc.vector.bn_aggr(out=mv, in_=stats)
                        sd = stC.tile([128, 1], F32, name="ln_sd", tag="ln_sd")
                        nc.scalar.activation(sd, mv[:, 1:2], AF.Sqrt,
                                             bias=eps_ln)
                        rstd = stC.tile([128, 1], F32, name="ln_rs",
                                        tag="ln_rs")
                        nc.vector.reciprocal(rstd, sd)
                        xln = stC.tile([128, D], F32, name="ln_x", tag="ln_x")
                        nc.vector.tensor_scalar(
                            out=xln, in0=xnew[j], scalar1=mv[:, 0:1],
                            scalar2=rstd, op0=ALU.subtract, op1=ALU.mult)
                        nc.vector.tensor_mul(xln, xln, gamma_rep)
                        nc.vector.tensor_add(xln, xln, beta_rep)
                        nc.vector.tensor_add(x_own[j], x_own[j], xln)

                    # C9: AllGather x, rebuild xT / x_ownT
                    xown_dram = dram.tile([NO, D], F32, name=f"xo_dram{l}")
                    xfull_dram = dram.tile([n, D], F32, name=f"xf_dram{l}",
                                           addr_space="Shared")
                    for j in range(OB):
                        nc.sync.dma_start(
                            out=xown_dram[j * 128:(j + 1) * 128, :],
                            in_=x_own[j])
                    nc.gpsimd.collective_compute(
                        "AllGather", ALU.bypass, replica_groups=rg,
                        ins=[xown_dram[:]], outs=[xfull_dram[:]])
                    with tc.tile_pool(name=f"C9ps{l}", bufs=2,
                                      space="PSUM") as c9ps:
                        for b in range(NB):
                            xf = stC.tile([128, D], F32, name="xf", tag="xf")
                            nc.sync.dma_start(
                                out=xf,
                                in_=xfull_dram[b * 128:(b + 1) * 128, :])
                            for k in range(2):
                                transpose_into(xT[k][:, b * 128:(b + 1) * 128],
                                               xf[:, k * 128:(k + 1) * 128],
                                               c9ps, F32, tag="C9")
                        for j in range(OB):
                            for k in range(2):
                                transpose_into(
                                    x_ownT[k][:, j * 128:(j + 1) * 128],
                                    x_own[j][:, k * 128:(k + 1) * 128],
                                    c9ps, F32, tag="C9")

        # =================================================================
        # Stage D: messages -> dec -> MHA -> gate -> output
        # =================================================================
        with contextlib.ExitStack() as dst:
            stD = dst.enter_context(tc.tile_pool(name="stD", bufs=2))
            stD_ps = dst.enter_context(
                tc.tile_pool(name="stD_ps", bufs=3, space="PSUM"))
            dper = dst.enter_context(tc.tile_pool(name="dper", bufs=1))

            def colvec(src_ap, parts, name):
                t = dper.tile([parts, 1], F32, name=name, tag=name)
                nc.sync.dma_start(out=t, in_=src_ap.rearrange("(p o) -> p o", o=1))
                return t

            def load_f32(src_ap, shape, name):
                t = dper.tile(list(shape), F32, name=name, tag=name)
                nc.sync.dma_start(out=t, in_=src_ap)
                return t

            def load_bf(src_ap, shape, name):
                f = stD.tile(list(shape), F32, name=name + "_f", tag="wload")
                nc.sync.dma_start(out=f, in_=src_ap)
                b = dper.tile(list(shape), BF16, name=name, tag=name)
                nc.vector.tensor_copy(b, f)
                return b

            encW1_f = [load_f32(enc_W1[k * 128:(k + 1) * 128, :], (128, 2 * MSG),
                                f"encW1_{k}") for k in range(2)]
            encW2_bf = load_bf(enc_W2[:], (2 * MSG, MSG), "encW2")
            decW1_bf = load_bf(dec_W1[:], (MSG, D // 2), "decW1")
            decW2_bf = load_bf(dec_W2[:], (D // 2, D), "decW2")
            mhaw_bf = [load_bf(mha_in_w[k * 128:(k + 1) * 128, :], (128, 3 * D),
                               f"mhaw_{k}") for k in range(2)]
            encb1 = colvec(enc_b1[:], 2 * MSG, "encb1")
            encb2 = colvec(enc_b2[:], MSG, "encb2")
            decb1 = colvec(dec_b1[:], D // 2, "decb1")
            decb2 = [colvec(dec_b2[k * 128:(k + 1) * 128], 128, f"decb2_{k}")
                     for k in range(2)]
            mhab = [[colvec(mha_in_b[q * D + h * C:q * D + (h + 1) * C], C,
                            f"mhab{q}_{h}") for h in range(H)]
                    for q in range(2)]
            outb = [colvec(mha_out_b[k * 128:(k + 1) * 128], 128, f"outb{k}")
                    for k in range(2)]
            gateb1 = [colvec(gate_b1[k * 128:(k + 1) * 128], 128, f"gateb1_{k}")
                      for k in range(2)]
            gateb2 = colvec(gate_b2[:], 1, "gateb2")
            projb = [colvec(proj_b[k * 128:(k + 1) * 128], 128, f"projb{k}")
                     for k in range(2)]
            outw_h = [load_f32(mha_out_w[h * C:(h + 1) * C, :], (C, D),
                               f"outw_h{h}") for h in range(H)]
            gateW1_sb = [load_f32(gate_W1[k * 128:(k + 1) * 128, :], (128, D),
                                  f"gw1_{k}") for k in range(4)]
            gateW2_sb = [load_f32(gate_W2[k * 128:(k + 1) * 128, :], (128, 1),
                                  f"gw2_{k}") for k in range(2)]
            projW_sb = [load_f32(proj_W[k * 128:(k + 1) * 128, :], (128, D),
                                 f"pw{k}") for k in range(2)]
            Wv_aug = [dper.tile([128, H, C + 1], BF16, name=f"wva{k}",
                                tag=f"wva{k}") for k in range(2)]
            for k in range(2):
                nc.vector.memset(Wv_aug[k], 0.0)
                nc.vector.tensor_copy(
                    Wv_aug[k][:, :, 0:C],
                    mhaw_bf[k][:, 2 * D:3 * D].rearrange("p (h c) -> p h c",
                                                         h=H))
            bv_rep = [dper.tile([128, C + 1], F32, name=f"bvr{h}", tag=f"bvr{h}")
                      for h in range(H)]
            for h in range(H):
                nc.vector.memset(bv_rep[h], 1.0)
                nc.sync.dma_start(
                    out=bv_rep[h][:, 0:C],
                    in_=bcast_ap(mha_in_b[2 * D + h * C:2 * D + (h + 1) * C],
                                 128))

            def chainT(name, lhsT_tiles, rhs_tiles, p_out, width, func,
                       bias_col, pool, tag=None):
                res = pool.tile([p_out, width], BF16, name=name,
                                tag=tag or name, bufs=2 if tag else 1)
                for jb in range((width + 511) // 512):
                    w = min(512, width - jb * 512)
                    ps = stD_ps.tile([p_out, 512], F32, name=name + "_ps",
                                     tag="D_ps")
                    nk = len(lhsT_tiles)
                    for k in range(nk):
                        nc.tensor.matmul(
                            ps[:, 0:w], lhsT_tiles[k],
                            rhs_tiles[k][:, jb * 512:jb * 512 + w],
                            start=(k == 0), stop=(k == nk - 1))
                    nc.scalar.activation(res[:, jb * 512:jb * 512 + w],
                                         ps[:, 0:w], func, bias=bias_col)
                return res

            # D1-D4: message/dec chain, transposed, full N.
            # m1T/mT/d1T share one 2-slot tag; decT persists for kT/v.
            m1T = chainT("m1T", encW1_f, xT, 2 * MSG, n, AF.Relu, encb1, dper,
                         tag="dchain")
            mT = chainT("mT", [encW2_bf], [m1T], MSG, n, AF.Identity, encb2,
                        dper, tag="dchain")
            d1T = chainT("d1T", [decW1_bf], [mT], D // 2, n, AF.Relu, decb1,
                         dper, tag="dchain")
            decT = [chainT(f"decT{m}", [decW2_bf[:, m * 128:(m + 1) * 128]],
                           [d1T], 128, n, AF.Identity, decb2[m], dper)
                    for m in range(2)]
            # D5: own chain (for q)
            m1To = chainT("m1To", encW1_f, x_ownT, 2 * MSG, NO, AF.Relu, encb1,
                          stD)
            # xT and x_ownT have no consumers after this point
            xT_stack.close()
            mTo = chainT("mTo", [encW2_bf], [m1To], MSG, NO, AF.Identity,
                         encb2, stD)
            d1To = chainT("d1To", [decW1_bf], [mTo], D // 2, NO, AF.Relu,
                          decb1, stD)
            decTo = [chainT(f"decTo{m}", [decW2_bf[:, m * 128:(m + 1) * 128]],
                            [d1To], 128, NO, AF.Identity, decb2[m], stD)
                     for m in range(2)]
            qTo = [chainT(f"qTo{h}",
                          [mhaw_bf[k][:, h * C:(h + 1) * C] for k in range(2)],
                          decTo, C, NO, AF.Identity, mhab[0][h], stD)
                   for h in range(H)]

            # D9: attention per head (k^T and v built per-head to bound SBUF)
            ctxcat = [stD.tile([C, NO], F32, name=f"ctx{h}", tag=f"ctx{h}",
                               bufs=1) for h in range(H)]
            sden_dram = dram.tile([H, NO], F32, name="sden_dram")
            inv_sqrt_hd = 1.0 / math.sqrt(C)
            with tc.tile_pool(name="mha_ps", bufs=1, space="PSUM") as mps, \
                 tc.tile_pool(name="mha_sb", bufs=3) as msb, \
                 tc.tile_pool(name="mha_kv", bufs=1) as mkv:
                for h in range(H):
                    kTh = chainT(f"kT{h}",
                                 [mhaw_bf[k][:, D + h * C:D + (h + 1) * C]
                                  for k in range(2)],
                                 decT, C, n, AF.Identity, mhab[1][h], mkv,
                                 tag="kTh")
                    v_h = []
                    for b in range(NB):
                        ps = stD_ps.tile([128, C + 1], F32, name="vps",
                                         tag="D_ps")
                        for k in range(2):
                            nc.tensor.matmul(
                                ps, decT[k][:, b * 128:(b + 1) * 128],
                                Wv_aug[k][:, h, :], start=(k == 0),
                                stop=(k == 1))
                        vt = mkv.tile([128, C + 1], BF16, name=f"v_{b}",
                                      tag=f"v_{b}", bufs=2)
                        nc.vector.tensor_add(vt, ps, bv_rep[h])
                        v_h.append(vt)
                    cps = mps.tile([C + 1, NO], F32, name="cps", tag="cps")
                    for b in range(NB):
                        sps = mps.tile([128, NO], F32, name="sps", tag="sps",
                                       bufs=2)
                        nc.tensor.matmul(sps, kTh[:, b * 128:(b + 1) * 128],
                                         qTo[h], start=True, stop=True)
                        pt = msb.tile([128, NO], BF16, name="pt", tag="pt")
                        nc.scalar.activation(pt, sps, AF.Exp,
                                             scale=inv_sqrt_hd)
                        nc.tensor.matmul(cps, v_h[b], pt,
                                         start=(b == 0), stop=(b == NB - 1))
                    rden = msb.tile([C + 1, NO], F32, name="rden", tag="rden",
                                    bufs=1)
                    nc.vector.reciprocal(rden[C:C + 1, :], cps[C:C + 1, :])
                    nc.sync.dma_start(out=sden_dram[h, :],
                                      in_=rden[C:C + 1, :])
                    rrep = msb.tile([C, NO], F32, name="rrep", tag="rrep",
                                    bufs=1)
                    nc.sync.dma_start(out=rrep,
                                      in_=bcast_ap(sden_dram[h, :], C))
                    nc.vector.tensor_mul(ctxcat[h], cps[0:C, :], rrep)

            # D10: agg^T = out_w^T @ ctx + bias
            aggT = [stD.tile([128, NO], F32, name=f"aggT{m}", tag=f"aggT{m}",
                             bufs=1) for m in range(2)]
            for m in range(2):
                ps = stD_ps.tile([128, NO], F32, name="aggps", tag="D_ps")
                for h in range(H):
                    nc.tensor.matmul(ps, outw_h[h][:, m * 128:(m + 1) * 128],
                                     ctxcat[h], start=(h == 0),
                                     stop=(h == H - 1))
                nc.scalar.activation(aggT[m], ps, AF.Identity, bias=outb[m])

            # D11: gate MLP (transposed)
            ginT = states_ownT + aggT
            g1T = []
            for m in range(2):
                ps = stD_ps.tile([128, NO], F32, name="g1ps", tag="D_ps")
                for k in range(4):
                    nc.tensor.matmul(ps,
                                     gateW1_sb[k][:, m * 128:(m + 1) * 128],
                                     ginT[k], start=(k == 0), stop=(k == 3))
                g1 = stD.tile([128, NO], F32, name=f"g1T{m}", tag=f"g1T{m}",
                              bufs=1)
                nc.scalar.activation(g1, ps, AF.Relu, bias=gateb1[m])
                g1T.append(g1)
            sps1 = stD_ps.tile([1, NO], F32, name="sps1", tag="D_ps")
            for k in range(2):
                nc.tensor.matmul(sps1, gateW2_sb[k], g1T[k],
                                 start=(k == 0), stop=(k == 1))
            s_row = stD.tile([1, NO], F32, name="s_row", tag="s_row", bufs=1)
            nc.scalar.activation(s_row, sps1, AF.Sigmoid, bias=gateb2)
            s_dram = dram.tile([NO], F32, name="s_dram")
            nc.sync.dma_start(out=s_dram[:], in_=s_row)
            s_rep = stD.tile([128, NO], F32, name="s_rep", tag="s_rep", bufs=1)
            nc.sync.dma_start(out=s_rep, in_=bcast_ap(s_dram[:], 128))

            # D13: out^T = (proj_W^T @ agg^T) * s + proj_b + states^T
            outT = [stD.tile([128, NO], F32, name=f"outT{m}", tag=f"outT{m}",
                             bufs=1) for m in range(2)]
            for m in range(2):
                ps = stD_ps.tile([128, NO], F32, name="pops", tag="D_ps")
                for k in range(2):
                    nc.tensor.matmul(ps, projW_sb[k][:, m * 128:(m + 1) * 128],
                                     aggT[k], start=(k == 0), stop=(k == 1))
                t = stD.tile([128, NO], F32, name="po_t", tag="po_t")
                nc.vector.tensor_mul(t, ps, s_rep)
                nc.scalar.activation(t, t, AF.Identity, bias=projb[m])
                nc.vector.tensor_add(outT[m], t, states_ownT[m])

            # D14: transpose to natural layout and store
            with tc.tile_pool(name="D14ps", bufs=2, space="PSUM") as dps:
                for j in range(OB):
                    on = stD.tile([128, D], F32, name="on", tag="on")
                    for m in range(2):
                        transpose_into(on[:, m * 128:(m + 1) * 128],
                                       outT[m][:, j * 128:(j + 1) * 128],
                                       dps, F32, tag="D14")
                    nc.sync.dma_start(out=out_own[j * 128:(j + 1) * 128, :],
                                      in_=on)

    nc.compile()
    return nc


_CACHED = {}


def _get_program(n=N, ncores=NCORES):
    key = (n, ncores)
    if key not in _CACHED:
        _CACHED[key] = build_program(n, ncores)
    return _CACHED[key]


def make_in_maps(inputs, n, ncores):
    no = n // ncores
    full = {k: np.ascontiguousarray(np.asarray(v, np.float32))
            for k, v in inputs.items()}
    in_maps = []
    for c in range(ncores):
        m = dict(full)
        sl = slice(c * no, (c + 1) * no)
        m["states_own"] = full["agent_states"][sl]
        m["emb_own"] = full["agent_emb"][sl]
        m["role_own"] = full["role_emb"][sl]
        in_maps.append(m)
    return in_maps


def run(inputs, trace=False, trace_cores=None):
    n = inputs["agent_states"].shape[0]
    ncores = NCORES
    nc = _get_program(n, ncores)
    in_maps = make_in_maps(inputs, n, ncores)
    kwargs = {}
    if trace:
        install_ntff_hook()
        kwargs = dict(trace=True, trace_cores=trace_cores or [0])
    res = bass_utils.run_bass_kernel_spmd(
        nc, in_maps, core_ids=list(range(ncores)), **kwargs)
    out = np.concatenate([res.results[c]["out_own"] for c in range(ncores)],
                         axis=0)
    return out.astype(np.float32), res


def kernel(**inputs) -> np.ndarray:
    return run(inputs)[0]
